# revision 2
# baseline (speedup 1.0000x reference)
"""Trainium2 Bass kernel for nn_Attention_39573828665647 — v2 (restructured).

GQA causal attention block (B=4, S=1024, DIM=2048, 32 q heads / 8 kv heads,
hd=64) with RoPE, sharded over 8 NeuronCores as (batch x head-half):
core = 2*b + hh handles batch b and kv groups [4hh, 4hh+4) (16 q heads).
Each core computes a partial output projection over its 1024 o-dims; the
host sums the two partials per batch.

v2 changes vs v1:
  - all PE operands bf16 (halves DMA + SBUF; 1 cycle/row at any moving size)
  - kv projected FIRST with d-outer accumulation (PE streams with the DMA
    fill instead of stalling on a 12MB prefetch)
  - q projection interleaved with per-head attention: projection matmuls
    fill PE while ACT runs exp, killing the attention-phase PE idle
  - scores j+1 emitted before AV j (1-deep software pipeline per head)
  - AV stationary carries 64 ones-columns so the softmax denominator lands
    replicated on psum partitions 64:128 -> reciprocal+mul directly, no
    DRAM-bounce partition broadcast
"""

from contextlib import ExitStack
import numpy as np

B, S, DIM = 4, 1024, 2048
NH, NKV, HD = 32, 8, 64
P = 128
ND = DIM // P  # 16 d-tiles

_SWAP_ADJ = [i ^ 1 for i in range(32)]  # pairwise partition swap within quadrants

_CACHE = {}


def host_prep(x, freqs_cos, freqs_sin, wqkv, wo):
    """Build the 8 per-core input dicts (bf16 weights/activations)."""
    import ml_dtypes
    bf16 = ml_dtypes.bfloat16
    x = np.asarray(x, np.float32)
    wqkv = np.asarray(wqkv, np.float32)
    wo = np.asarray(wo, np.float32)
    cos = np.asarray(freqs_cos, np.float32)
    sin = np.asarray(freqs_sin, np.float32)

    cosT, sinT = cos.T, sin.T                      # [32, S]
    C64 = np.repeat(cosT, 2, axis=0)               # [64, S]
    Ss64 = np.repeat(sinT, 2, axis=0).copy()
    Ss64[0::2] *= -1.0                             # even rows -sin, odd +sin
    scale = np.float32(1.0 / np.sqrt(HD))
    Cq = np.ascontiguousarray(C64 * scale, np.float32)
    Sq = np.ascontiguousarray(Ss64 * scale, np.float32)
    Ck = np.ascontiguousarray(C64, np.float32)
    Sk = np.ascontiguousarray(Ss64, np.float32)

    woT_full = np.ascontiguousarray(wo.T)                 # [d', o]
    xT_full = np.ascontiguousarray(
        x.transpose(0, 2, 1)).astype(bf16)                # [B, DIM, S]
    wqkvT_full = np.ascontiguousarray(wqkv.T)             # [DIM, 3072]
    in_maps = []
    for core in range(8):
        b, hh = core // 2, core % 2
        groups = range(4 * hh, 4 * hh + 4)
        qheads = range(16 * hh, 16 * hh + 16)
        # assemble wqkvT from contiguous 64-column blocks: 16 q heads,
        # then 4 k groups, then 4 v groups
        wqkvT = np.empty((DIM, 1536), np.float32)
        col = 0
        blocks = ([(h // 4 * 6 + h % 4) * 64 for h in qheads]
                  + [(g * 6 + 4) * 64 for g in groups]
                  + [(g * 6 + 5) * 64 for g in groups])
        for c0 in blocks:
            wqkvT[:, col:col + 64] = wqkvT_full[:, c0:c0 + 64]
            col += 64
        in_maps.append({
            "xT": xT_full[b],                             # [2048, 1024] bf16
            "wqkvT": wqkvT.astype(bf16),                  # [2048, 1536] bf16
            "woT": np.ascontiguousarray(
                woT_full[1024 * hh:1024 * hh + 1024]).astype(bf16),
            "Cq": Cq, "Sq": Sq, "Ck": Ck, "Sk": Sk,
        })
    return in_maps


def build_nc(reps=1):
    import concourse.bacc as bacc
    import concourse.bass as bass
    import concourse.tile as tile
    import concourse.mybir as mybir

    f32 = mybir.dt.float32
    bf16 = mybir.dt.bfloat16
    EXP = mybir.ActivationFunctionType.Exp

    nc = bacc.Bacc("TRN2", target_bir_lowering=False, debug=False)
    xT_d = nc.dram_tensor("xT", [DIM, S], bf16, kind="ExternalInput")
    wqkvT_d = nc.dram_tensor("wqkvT", [DIM, 1536], bf16, kind="ExternalInput")
    woT_d = nc.dram_tensor("woT", [1024, DIM], bf16, kind="ExternalInput")
    Cq_d = nc.dram_tensor("Cq", [64, S], f32, kind="ExternalInput")
    Sq_d = nc.dram_tensor("Sq", [64, S], f32, kind="ExternalInput")
    Ck_d = nc.dram_tensor("Ck", [64, S], f32, kind="ExternalInput")
    Sk_d = nc.dram_tensor("Sk", [64, S], f32, kind="ExternalInput")
    out_d = nc.dram_tensor("out", [S, DIM], f32, kind="ExternalOutput")

    def emit(tc, pfx):
        with ExitStack() as stack:
            resid = stack.enter_context(tc.tile_pool(name=pfx + "resid", bufs=1))

            def rtile(shape, dt_, nm):
                return resid.tile(shape, dt_, tag=pfx + nm, name=pfx + nm)

            q_sb = [rtile([P, S], bf16, f"q{i}") for i in range(8)]
            # k pair-layout: kk[blk] rows 0:64 = group 2blk, 64:128 = 2blk+1
            # (RoPE writes it directly); kk2[blk] is the swapped duplicate
            kk = [rtile([P, S], bf16, f"kk{b}") for b in range(2)]
            kk2 = [rtile([P, S], bf16, f"kk2{b}") for b in range(2)]
            vaug = [rtile([P, 4, 128], bf16, f"va{i}") for i in range(8)]
            o_sb = [rtile([P, S], bf16, f"o{i}") for i in range(8)]

            def kh_ap(g, par):
                blk = g // 2
                tile = (kk if (g % 2 == 0) == (par == 0) else kk2)[blk]
                return tile[par * 64:par * 64 + 64, :]

            rc_pool = stack.enter_context(
                tc.tile_pool(name=pfx + "ropeconst", bufs=1))
            rt_pool = stack.enter_context(
                tc.tile_pool(name=pfx + "ropetmp", bufs=2))

            # ---------------- DMA schedule + phase 0 (kv, d-outer) --------
            # Batched DMAs: the HWDGE charges ~625ns of serialized queue time
            # per DMA instruction, so inputs load as a few large strided
            # transfers into big 3D tiles instead of per-[128,512] tiles.
            c_sb = {}

            def dram_chunk(dr, row0, nrows_d, col0, ncols, row_stride):
                base = dr[0:1, 0:1]
                return bass.AP(
                    tensor=base.tensor, offset=row0 * row_stride + col0,
                    ap=[[row_stride, P], [P * row_stride, nrows_d],
                        [1, ncols]])

            xbig = resid.tile([P, ND, S], bf16, tag=pfx + "xbig",
                              name=pfx + "xbig")
            wq1 = resid.tile([P, ND, 512], bf16, tag=pfx + "wq1",
                             name=pfx + "wq1")
            wq2 = resid.tile([P, ND, 512], bf16, tag=pfx + "wq2",
                             name=pfx + "wq2")

            # psQ reserved up front so qproj(0) never waits on a phase-0 bank
            psQ = stack.enter_context(
                tc.tile_pool(name=pfx + "psumQ", bufs=2, space="PSUM"))

            def emit_qproj(at, t):
                wtiles = wq1 if at < 4 else wq2
                coff = (at % 4) * P
                sl = slice(t * 512, (t + 1) * 512)
                pq = psQ.tile([P, 512], f32, tag="q",
                              name=pfx + f"pq{at}_{t}")
                for d in range(ND):
                    nc.tensor.matmul(
                        pq[:], wtiles[:, d, coff:coff + P],
                        xbig[:, d, sl], start=(d == 0), stop=(d == ND - 1))
                sh = rt_pool.tile([P, 512], f32, tag="sh",
                                  name=pfx + f"qsh{at}_{t}")
                m1 = rt_pool.tile([P, 512], f32, tag="m1",
                                  name=pfx + f"qm1{at}_{t}")
                m2 = rt_pool.tile([P, 512], f32, tag="m2",
                                  name=pfx + f"qm2{at}_{t}")
                nc.vector.stream_shuffle(sh[:], pq[:], _SWAP_ADJ)
                nc.vector.tensor_mul(m1[:], pq[:], c_sb["Cq"][:, sl])
                nc.gpsimd.tensor_mul(m2[:], sh[:], c_sb["Sq"][:, sl])
                nc.gpsimd.tensor_add(q_sb[at][:, sl], m1[:], m2[:])

            with tc.tile_pool(name=pfx + "wkv", bufs=1) as wkv_pool, \
                 tc.tile_pool(name=pfx + "psK", bufs=4, space="PSUM") as psK, \
                 tc.tile_pool(name=pfx + "psV", bufs=2, space="PSUM") as psV:

                wkvbig = wkv_pool.tile([P, ND, 512], bf16, tag="wkv",
                                       name=pfx + "wkv")

                # warmup spin: ramp PE + cover the first DMAs' latency
                wmt = rt_pool.tile([P, P], bf16, tag="wm", name=pfx + "wm")
                nc.vector.memset(wmt[:], 0.0)
                kp = [psK.tile([P, 512], f32, tag="kp", name=pfx + f"kp{i}")
                      for i in range(4)]
                for w in range(36):
                    nc.tensor.matmul(kp[0][:, 0:P], wmt[:], wmt[:],
                                     start=True, stop=True)

                # Input stream on 3 DMA queues, in consumption order:
                # x+wkv (1-/2-d chunks) -> rope tables -> wq1 -> wq2.
                qrr = [nc.sync, nc.scalar]
                qi = [0]

                def nextq():
                    qi[0] += 1
                    return qrr[qi[0] % 2]

                for dd in range(ND // 2):
                    nextq().dma_start(
                        out=xbig[:, 2 * dd:2 * dd + 1, :],
                        in_=dram_chunk(xT_d, 2 * dd * P, 1, 0, S, S))
                    nextq().dma_start(
                        out=xbig[:, 2 * dd + 1:2 * dd + 2, :],
                        in_=dram_chunk(xT_d, (2 * dd + 1) * P, 1, 0, S, S))
                    nextq().dma_start(
                        out=wkvbig[:, 2 * dd:2 * dd + 2, :],
                        in_=dram_chunk(wqkvT_d, 2 * dd * P, 2, 1024, 512,
                                       1536))
                # tables [64,S] f32; dup onto partitions 64:128 on idle DVE
                for nm, dr in (("Ck", Ck_d), ("Sk", Sk_d),
                               ("Cq", Cq_d), ("Sq", Sq_d)):
                    ct = rc_pool.tile([P, S], f32, tag=nm, name=pfx + nm)
                    nextq().dma_start(out=ct[0:64, :], in_=dr[:])
                    nc.vector.tensor_copy(ct[64:128, :], ct[0:64, :])
                    c_sb[nm] = ct
                for c in range(2):
                    nextq().dma_start(
                        out=wq1[:, 8 * c:8 * c + 8, :],
                        in_=dram_chunk(wqkvT_d, 8 * c * P, 8, 0, 512, 1536))
                for c in range(2):
                    nextq().dma_start(
                        out=wq2[:, 8 * c:8 * c + 8, :],
                        in_=dram_chunk(wqkvT_d, 8 * c * P, 8, 512, 512, 1536))

                # sweep 1: k (all 4 psums) + v st 0-3, d-outer with the fill
                vp = [psV.tile([P, 512], f32, tag="vp", name=pfx + f"vp{i}")
                      for i in range(2)]
                for d in range(ND):
                    for blk in (0, 1):
                        for t in (0, 1):
                            nc.tensor.matmul(
                                kp[blk * 2 + t][:],
                                wkvbig[:, d, blk * P:(blk + 1) * P],
                                xbig[:, d, t * 512:(t + 1) * 512],
                                start=(d == 0), stop=(d == ND - 1))
                    for st in range(4):
                        nc.tensor.matmul(
                            vp[st // 2][:, (st % 2) * 256:(st % 2) * 256 + 256],
                            xbig[:, d, st * P:(st + 1) * P],
                            wkvbig[:, d, 256:512],
                            start=(d == 0 and st % 2 == 0),
                            stop=(d == ND - 1 and st % 2 == 1))

                def copy_on(eng, dst, src):
                    if eng is nc.scalar:
                        eng.copy(dst, src)
                    else:
                        eng.tensor_copy(dst, src)

                def rope_k(blk, t):
                    # writes kk[blk] directly; kk2 gets the swapped duplicate
                    sl = slice(t * 512, (t + 1) * 512)
                    pk = kp[blk * 2 + t]
                    sh = rt_pool.tile([P, 512], f32, tag="sh",
                                      name=pfx + f"ksh{blk}_{t}")
                    m1 = rt_pool.tile([P, 512], f32, tag="m1",
                                      name=pfx + f"km1{blk}_{t}")
                    m2 = rt_pool.tile([P, 512], f32, tag="m2",
                                      name=pfx + f"km2{blk}_{t}")
                    nc.vector.stream_shuffle(sh[:], pk[:], _SWAP_ADJ)
                    nc.vector.tensor_mul(m1[:], pk[:], c_sb["Ck"][:, sl])
                    nc.gpsimd.tensor_mul(m2[:], sh[:], c_sb["Sk"][:, sl])
                    nc.gpsimd.tensor_add(kk[blk][:, sl], m1[:], m2[:])
                    nc.scalar.copy(kk2[blk][64:128, sl], kk[blk][0:64, sl])
                    nc.vector.tensor_copy(kk2[blk][0:64, sl],
                                          kk[blk][64:128, sl])

                def spread_v(vtiles, st):
                    base = (st % 2) * 256
                    for g in range(4):
                        eng = nc.scalar if g % 2 == 0 else nc.vector
                        copy_on(eng, vaug[st][:, g, 0:64],
                                vtiles[(st % 4) // 2]
                                [:, base + g * 64:base + g * 64 + 64])
                    nc.vector.memset(vaug[st][:, :, 64:128], 1.0)

                # engine-side work, priority order: k for pair 0 first, then
                # the v banks sweep 2 reuses, then the rest
                rope_k(0, 0)
                for st in (0, 1):
                    spread_v(vp, st)
                rope_k(0, 1)
                for st in (2, 3):
                    spread_v(vp, st)
                rope_k(1, 0)
                rope_k(1, 1)

                # PE: qproj(0,0) before sweep 2 (wq1 lands before wq2)
                emit_qproj(0, 0)

                # sweep 2: v st 4-7 (reuses the two psV banks)
                vp2 = [psV.tile([P, 512], f32, tag="vp", name=pfx + f"vp2_{i}")
                       for i in range(2)]
                for d in range(ND):
                    for st in range(4, 8):
                        nc.tensor.matmul(
                            vp2[(st - 4) // 2]
                            [:, (st % 2) * 256:(st % 2) * 256 + 256],
                            xbig[:, d, st * P:(st + 1) * P],
                            wkvbig[:, d, 256:512],
                            start=(d == 0 and st % 2 == 0),
                            stop=(d == ND - 1 and st % 2 == 1))
                for st in (4, 5, 6, 7):
                    spread_v(vp2, st)
                emit_qproj(0, 1)

            # ---------------- phases 1+2 + stage E ------------------------
            wo_pool = stack.enter_context(tc.tile_pool(name=pfx + "wo", bufs=1))
            wobig = wo_pool.tile([P, 4, 8, 512], bf16, tag="wo",
                                 name=pfx + "wo")

            def load_wo(ot):
                nc.sync.dma_start(
                    out=wobig[:, ot, :, :],
                    in_=dram_chunk(woT_d, 0, 8, ot * 512, 512, DIM))

            with tc.tile_pool(name=pfx + "expT", bufs=8) as e_pool, \
                 tc.tile_pool(name=pfx + "normtmp", bufs=3) as n_pool, \
                 tc.tile_pool(name=pfx + "outsb", bufs=4) as ob_pool, \
                 tc.tile_pool(name=pfx + "psumS", bufs=3, space="PSUM") as psS, \
                 tc.tile_pool(name=pfx + "psumO", bufs=3, space="PSUM") as psO:

                def normalize(qt, par, t, op):
                    rcp = n_pool.tile([64, 512], f32, tag="rcp",
                                      name=pfx + f"rcp{qt}_{par}_{t}")
                    nc.vector.reciprocal(rcp[:], op[64:128, :])
                    nc.vector.tensor_mul(
                        o_sb[qt][par * 64:par * 64 + 64,
                                 t * 512:(t + 1) * 512],
                        op[0:64, :], rcp[:])

                def emit_pair_pass(qt, t):
                    """Both heads of q-tile qt, sq half t, j-interleaved.

                    Pass t=0 covers sq [0:512] (sk tiles j=0..3); pass t=1
                    covers sq [512:1024] (all 8 sk tiles). Scores for step
                    j+1 are emitted before AV of step j so ACT's exp hides
                    behind PE work.
                    """
                    g = qt // 2
                    njs = 4 if t == 0 else 8
                    op = {}
                    for par in (0, 1):
                        op[par] = psO.tile([P, 512], f32, tag="op",
                                           name=pfx + f"op{qt}_{par}_{t}")
                    pend = []
                    for j in range(njs):
                        lo = j * P           # sk tile start (global)
                        ll = max(lo - 512 * t, 0)   # local col of sq>=sk edge
                        new = []
                        for par in (0, 1):
                            qh = q_sb[qt][par * 64:par * 64 + 64, :]
                            kh = kh_ap(g, par)
                            sp = psS.tile([P, 512], f32, tag="sp",
                                          name=pfx + f"sp{qt}_{par}_{t}_{j}")
                            nc.tensor.matmul(
                                sp[:, ll:512], kh[:, lo:lo + P],
                                qh[:, 512 * t + ll:512 * (t + 1)],
                                start=True, stop=True)
                            et = e_pool.tile([P, 512], bf16, tag="et",
                                             name=pfx + f"et{qt}_{par}_{t}_{j}")
                            nc.scalar.activation(et[:, ll:512], sp[:, ll:512],
                                                 EXP)
                            if lo >= 512 * t:  # diagonal chunk in this pass
                                nc.gpsimd.affine_select(
                                    out=et[:, ll:ll + P], in_=et[:, ll:ll + P],
                                    pattern=[[1, P]], channel_multiplier=-1,
                                    base=0, compare_op=mybir.AluOpType.is_ge,
                                    fill=0.0)
                            new.append((par, j, ll, et))
                        for par, jj, jl, et in pend:
                            nc.tensor.matmul(
                                op[par][:, jl:512], vaug[jj][:, g, :],
                                et[:, jl:512],
                                start=(jj == 0), stop=(jj == njs - 1))
                        pend = new
                    for par, jj, jl, et in pend:
                        nc.tensor.matmul(
                            op[par][:, jl:512], vaug[jj][:, g, :],
                            et[:, jl:512],
                            start=(jj == 0), stop=(jj == njs - 1))
                    for par in (0, 1):
                        normalize(qt, par, t, op[par])

                # phase 2: q projection interleaved with attention
                for qt in range(8):
                    if qt < 7:
                        emit_qproj(qt + 1, 0)
                    emit_pair_pass(qt, 0)
                    if qt < 7:
                        emit_qproj(qt + 1, 1)
                    emit_pair_pass(qt, 1)
                    if qt < 4:
                        load_wo(qt)

                # ------------- Stage E: output projection -------------
                for ot in range(4):
                    for sc in range(8):
                        pe = psO.tile([P, 512], f32, tag="op",
                                      name=pfx + f"pe{ot}_{sc}")
                        for dt_ in range(8):
                            nc.tensor.matmul(
                                pe[:], o_sb[dt_][:, sc * P:(sc + 1) * P],
                                wobig[:, ot, dt_, :],
                                start=(dt_ == 0), stop=(dt_ == 7))
                        ob = ob_pool.tile([P, 512], f32, tag="ob",
                                          name=pfx + f"ob{ot}_{sc}")
                        nc.vector.tensor_copy(ob[:], pe[:])
                        (nc.sync if sc % 2 else nc.scalar).dma_start(
                            out=out_d[sc * P:(sc + 1) * P,
                                      ot * 512:(ot + 1) * 512],
                            in_=ob[:])

    import concourse.tile as tile_mod
    with tile_mod.TileContext(nc) as tc:
        for rep in range(reps):
            emit(tc, f"r{rep}_" if reps > 1 else "")

    nc.compile()
    return nc


def _get_nc():
    if "nc" not in _CACHE:
        _CACHE["nc"] = build_nc()
    return _CACHE["nc"]


def kernel(**inputs):
    from concourse.bass_utils import run_bass_kernel_spmd
    nc = _get_nc()
    in_maps = host_prep(**inputs)
    res = run_bass_kernel_spmd(nc, in_maps, core_ids=list(range(8)))
    outs = [res.results[c]["out"] for c in range(8)]
    full = np.stack([outs[2 * b] + outs[2 * b + 1] for b in range(B)])
    return full.astype(np.float32)


if __name__ == "__main__":
    nc = build_nc()
    print("build ok")


# revision 4
# speedup vs baseline: 1.1297x; 1.1297x over previous
"""Trainium2 Bass kernel for nn_Attention_39573828665647 — v2 (restructured).

GQA causal attention block (B=4, S=1024, DIM=2048, 32 q heads / 8 kv heads,
hd=64) with RoPE, sharded over 8 NeuronCores as (batch x head-half):
core = 2*b + hh handles batch b and kv groups [4hh, 4hh+4) (16 q heads).
Each core computes a partial output projection over its 1024 o-dims; the
host sums the two partials per batch.

v2 changes vs v1:
  - all PE operands bf16 (halves DMA + SBUF; 1 cycle/row at any moving size)
  - kv projected FIRST with d-outer accumulation (PE streams with the DMA
    fill instead of stalling on a 12MB prefetch)
  - q projection interleaved with per-head attention: projection matmuls
    fill PE while ACT runs exp, killing the attention-phase PE idle
  - scores j+1 emitted before AV j (1-deep software pipeline per head)
  - AV stationary carries 64 ones-columns so the softmax denominator lands
    replicated on psum partitions 64:128 -> reciprocal+mul directly, no
    DRAM-bounce partition broadcast
"""

from contextlib import ExitStack
import numpy as np

B, S, DIM = 4, 1024, 2048
NH, NKV, HD = 32, 8, 64
P = 128
ND = DIM // P  # 16 d-tiles

_SWAP_ADJ = [i ^ 1 for i in range(32)]  # pairwise partition swap within quadrants

_CACHE = {}


def host_prep(x, freqs_cos, freqs_sin, wqkv, wo):
    """Build the 8 per-core input dicts (bf16 weights/activations)."""
    import ml_dtypes
    bf16 = ml_dtypes.bfloat16
    x = np.asarray(x, np.float32)
    wqkv = np.asarray(wqkv, np.float32)
    wo = np.asarray(wo, np.float32)
    cos = np.asarray(freqs_cos, np.float32)
    sin = np.asarray(freqs_sin, np.float32)

    cosT, sinT = cos.T, sin.T                      # [32, S]
    C64 = np.repeat(cosT, 2, axis=0)               # [64, S]
    Ss64 = np.repeat(sinT, 2, axis=0).copy()
    Ss64[0::2] *= -1.0                             # even rows -sin, odd +sin
    Ck = np.ascontiguousarray(C64, np.float32)
    Sk = np.ascontiguousarray(Ss64, np.float32)

    woT_full = np.ascontiguousarray(wo.T)                 # [d', o]
    xT_full = np.ascontiguousarray(
        x.transpose(0, 2, 1)).astype(bf16)                # [B, DIM, S]
    wqkvT_full = np.ascontiguousarray(wqkv.T)             # [DIM, 3072]
    in_maps = []
    for core in range(8):
        b, hh = core // 2, core % 2
        groups = range(4 * hh, 4 * hh + 4)
        qheads = range(16 * hh, 16 * hh + 16)
        # assemble wqkvT from contiguous 64-column blocks: 16 q heads,
        # then 4 k groups, then 4 v groups
        wqkvT = np.empty((DIM, 1536), np.float32)
        col = 0
        blocks = ([(h // 4 * 6 + h % 4) * 64 for h in qheads]
                  + [(g * 6 + 4) * 64 for g in groups]
                  + [(g * 6 + 5) * 64 for g in groups])
        for c0 in blocks:
            wqkvT[:, col:col + 64] = wqkvT_full[:, c0:c0 + 64]
            col += 64
        in_maps.append({
            "xT": xT_full[b],                             # [2048, 1024] bf16
            "wqkvT": wqkvT.astype(bf16),                  # [2048, 1536] bf16
            "woT": np.ascontiguousarray(
                woT_full[1024 * hh:1024 * hh + 1024]).astype(bf16),
            "Ck": Ck, "Sk": Sk,
        })
    return in_maps


def build_nc(reps=1):
    import concourse.bacc as bacc
    import concourse.bass as bass
    import concourse.tile as tile
    import concourse.mybir as mybir

    f32 = mybir.dt.float32
    bf16 = mybir.dt.bfloat16
    EXP = mybir.ActivationFunctionType.Exp

    nc = bacc.Bacc("TRN2", target_bir_lowering=False, debug=False)
    xT_d = nc.dram_tensor("xT", [DIM, S], bf16, kind="ExternalInput")
    wqkvT_d = nc.dram_tensor("wqkvT", [DIM, 1536], bf16, kind="ExternalInput")
    woT_d = nc.dram_tensor("woT", [1024, DIM], bf16, kind="ExternalInput")
    Ck_d = nc.dram_tensor("Ck", [64, S], f32, kind="ExternalInput")
    Sk_d = nc.dram_tensor("Sk", [64, S], f32, kind="ExternalInput")
    out_d = nc.dram_tensor("out", [S, DIM], f32, kind="ExternalOutput")

    def emit(tc, pfx):
        with ExitStack() as stack:
            resid = stack.enter_context(tc.tile_pool(name=pfx + "resid", bufs=1))

            def rtile(shape, dt_, nm):
                return resid.tile(shape, dt_, tag=pfx + nm, name=pfx + nm)

            q_sb = [rtile([P, S], bf16, f"q{i}") for i in range(8)]
            # k pair-layout: kk[blk] rows 0:64 = group 2blk, 64:128 = 2blk+1
            # (RoPE writes it directly); kk2[blk] is the swapped duplicate
            kk = [rtile([P, S], bf16, f"kk{b}") for b in range(2)]
            kk2 = [rtile([P, S], bf16, f"kk2{b}") for b in range(2)]
            vaug = [rtile([P, 4, 128], bf16, f"va{i}") for i in range(8)]
            o_sb = [rtile([P, S], bf16, f"o{i}") for i in range(8)]

            def kh_ap(g, par):
                blk = g // 2
                tile = (kk if (g % 2 == 0) == (par == 0) else kk2)[blk]
                return tile[par * 64:par * 64 + 64, :]

            rc_pool = stack.enter_context(
                tc.tile_pool(name=pfx + "ropeconst", bufs=1))
            rt_pool = stack.enter_context(
                tc.tile_pool(name=pfx + "ropetmp", bufs=2))

            # ---------------- DMA schedule + phase 0 (kv, d-outer) --------
            # Batched DMAs: the HWDGE charges ~625ns of serialized queue time
            # per DMA instruction, so inputs load as a few large strided
            # transfers into big 3D tiles instead of per-[128,512] tiles.
            c_sb = {}

            def dram_chunk(dr, row0, nrows_d, col0, ncols, row_stride):
                base = dr[0:1, 0:1]
                return bass.AP(
                    tensor=base.tensor, offset=row0 * row_stride + col0,
                    ap=[[row_stride, P], [P * row_stride, nrows_d],
                        [1, ncols]])

            xbig = resid.tile([P, ND, S], bf16, tag=pfx + "xbig",
                              name=pfx + "xbig")
            wq1 = resid.tile([P, ND, 512], bf16, tag=pfx + "wq1",
                             name=pfx + "wq1")
            wq2 = resid.tile([P, ND, 512], bf16, tag=pfx + "wq2",
                             name=pfx + "wq2")

            # psQ reserved up front so qproj(0) never waits on a phase-0 bank
            psQ = stack.enter_context(
                tc.tile_pool(name=pfx + "psumQ", bufs=2, space="PSUM"))

            def gen_qproj(at, t):
                """q-projection unit as a generator: 4-matmul chunks between
                yields so it can interleave as PE filler inside a pass."""
                wtiles = wq1 if at < 4 else wq2
                coff = (at % 4) * P
                sl = slice(t * 512, (t + 1) * 512)
                pq = psQ.tile([P, 512], f32, tag="q",
                              name=pfx + f"pq{at}_{t}")
                for c in range(4):
                    for d in range(4 * c, 4 * c + 4):
                        nc.tensor.matmul(
                            pq[:], wtiles[:, d, coff:coff + P],
                            xbig[:, d, sl], start=(d == 0),
                            stop=(d == ND - 1))
                    yield
                sh = rt_pool.tile([P, 512], f32, tag="sh",
                                  name=pfx + f"qsh{at}_{t}")
                m1 = rt_pool.tile([P, 512], f32, tag="m1",
                                  name=pfx + f"qm1{at}_{t}")
                m2 = rt_pool.tile([P, 512], f32, tag="m2",
                                  name=pfx + f"qm2{at}_{t}")
                nc.vector.stream_shuffle(sh[:], pq[:], _SWAP_ADJ)
                nc.vector.tensor_mul(m1[:], pq[:], c_sb["Ck"][:, sl])
                nc.gpsimd.tensor_mul(m2[:], sh[:], c_sb["Sk"][:, sl])
                nc.gpsimd.tensor_add(q_sb[at][:, sl], m1[:], m2[:])

            def emit_qproj(at, t):
                for _ in gen_qproj(at, t):
                    pass

            with tc.tile_pool(name=pfx + "wkv", bufs=1) as wkv_pool, \
                 tc.tile_pool(name=pfx + "psK", bufs=4, space="PSUM") as psK, \
                 tc.tile_pool(name=pfx + "psV", bufs=2, space="PSUM") as psV:

                wkvbig = wkv_pool.tile([P, ND, 512], bf16, tag="wkv",
                                       name=pfx + "wkv")

                # warmup spin: ramp PE + cover the first DMAs' latency
                wmt = rt_pool.tile([P, P], bf16, tag="wm", name=pfx + "wm")
                nc.vector.memset(wmt[:], 0.0)
                kp = [psK.tile([P, 512], f32, tag="kp", name=pfx + f"kp{i}")
                      for i in range(4)]
                for w in range(36):
                    nc.tensor.matmul(kp[0][:, 0:P], wmt[:], wmt[:],
                                     start=True, stop=True)

                # Input stream on 3 DMA queues, in consumption order:
                # x+wkv (1-/2-d chunks) -> rope tables -> wq1 -> wq2.
                qrr = [nc.sync, nc.scalar]
                qi = [0]

                def nextq():
                    qi[0] += 1
                    return qrr[qi[0] % 2]

                def copy_on(eng, dst, src):
                    if eng is nc.scalar:
                        eng.copy(dst, src)
                    else:
                        eng.tensor_copy(dst, src)

                for dd in range(ND // 2):
                    nextq().dma_start(
                        out=xbig[:, 2 * dd:2 * dd + 1, :],
                        in_=dram_chunk(xT_d, 2 * dd * P, 1, 0, S, S))
                    nextq().dma_start(
                        out=xbig[:, 2 * dd + 1:2 * dd + 2, :],
                        in_=dram_chunk(xT_d, (2 * dd + 1) * P, 1, 0, S, S))
                    nextq().dma_start(
                        out=wkvbig[:, 2 * dd:2 * dd + 2, :],
                        in_=dram_chunk(wqkvT_d, 2 * dd * P, 2, 1024, 512,
                                       1536))
                # tables [64,S] f32; dup onto partitions 64:128 on-chip.
                # k tables first (k-rope fires at sweep-1 end), dups on the
                # then-idle ACT; q tables after wq1, dups on DVE
                def table(nm, dr, eng):
                    ct = rc_pool.tile([P, S], f32, tag=nm, name=pfx + nm)
                    nextq().dma_start(out=ct[0:64, :], in_=dr[:])
                    copy_on(eng, ct[64:128, :], ct[0:64, :])
                    c_sb[nm] = ct

                table("Ck", Ck_d, nc.gpsimd)
                table("Sk", Sk_d, nc.gpsimd)
                for c in range(2):
                    nextq().dma_start(
                        out=wq1[:, 8 * c:8 * c + 8, :],
                        in_=dram_chunk(wqkvT_d, 8 * c * P, 8, 0, 512, 1536))
                for c in range(2):
                    nextq().dma_start(
                        out=wq2[:, 8 * c:8 * c + 8, :],
                        in_=dram_chunk(wqkvT_d, 8 * c * P, 8, 512, 512, 1536))

                # sweep 1: k (all 4 psums) + v st 0-3, d-outer with the fill
                vp = [psV.tile([P, 512], f32, tag="vp", name=pfx + f"vp{i}")
                      for i in range(2)]
                for d in range(ND):
                    for blk in (0, 1):
                        for t in (0, 1):
                            nc.tensor.matmul(
                                kp[blk * 2 + t][:],
                                wkvbig[:, d, blk * P:(blk + 1) * P],
                                xbig[:, d, t * 512:(t + 1) * 512],
                                start=(d == 0), stop=(d == ND - 1))
                    for st in range(4):
                        nc.tensor.matmul(
                            vp[st // 2][:, (st % 2) * 256:(st % 2) * 256 + 256],
                            xbig[:, d, st * P:(st + 1) * P],
                            wkvbig[:, d, 256:512],
                            start=(d == 0 and st % 2 == 0),
                            stop=(d == ND - 1 and st % 2 == 1))

                def rope_k(blk, t):
                    # writes kk[blk] directly; kk2 gets the swapped duplicate
                    sl = slice(t * 512, (t + 1) * 512)
                    pk = kp[blk * 2 + t]
                    sh = rt_pool.tile([P, 512], f32, tag="sh",
                                      name=pfx + f"ksh{blk}_{t}")
                    m1 = rt_pool.tile([P, 512], f32, tag="m1",
                                      name=pfx + f"km1{blk}_{t}")
                    m2 = rt_pool.tile([P, 512], f32, tag="m2",
                                      name=pfx + f"km2{blk}_{t}")
                    nc.vector.stream_shuffle(sh[:], pk[:], _SWAP_ADJ)
                    nc.vector.tensor_mul(m1[:], pk[:], c_sb["Ck"][:, sl])
                    nc.gpsimd.tensor_mul(m2[:], sh[:], c_sb["Sk"][:, sl])
                    nc.gpsimd.tensor_add(kk[blk][:, sl], m1[:], m2[:])
                    nc.scalar.copy(kk2[blk][64:128, sl], kk[blk][0:64, sl])
                    nc.vector.tensor_copy(kk2[blk][0:64, sl],
                                          kk[blk][64:128, sl])

                def spread_v(vtiles, st):
                    base = (st % 2) * 256
                    for g in range(4):
                        eng = nc.scalar if g % 2 == 0 else nc.vector
                        copy_on(eng, vaug[st][:, g, 0:64],
                                vtiles[(st % 4) // 2]
                                [:, base + g * 64:base + g * 64 + 64])
                    nc.vector.memset(vaug[st][:, :, 64:128], 1.0)

                # engine-side work, priority order: k for pair 0 first, then
                # the v banks sweep 2 reuses; rope_k(1,*) (groups 2-3, not
                # needed until qt=4) waits so q_sb[0]'s rope isn't queued
                # behind it on Pool/DVE
                rope_k(0, 0)
                for st in (0, 1):
                    spread_v(vp, st)
                rope_k(0, 1)
                for st in (2, 3):
                    spread_v(vp, st)

                # PE: qproj(0,0) d0-7, then sweep 2 while wq1's second DMA
                # chunk lands, then qproj(0,0) d8-15 (chain stays open)
                q00 = gen_qproj(0, 0)
                next(q00)
                next(q00)

                # sweep 2: v st 4-7 (reuses the two psV banks)
                vp2 = [psV.tile([P, 512], f32, tag="vp", name=pfx + f"vp2_{i}")
                       for i in range(2)]
                for d in range(ND):
                    for st in range(4, 8):
                        nc.tensor.matmul(
                            vp2[(st - 4) // 2]
                            [:, (st % 2) * 256:(st % 2) * 256 + 256],
                            xbig[:, d, st * P:(st + 1) * P],
                            wkvbig[:, d, 256:512],
                            start=(d == 0 and st % 2 == 0),
                            stop=(d == ND - 1 and st % 2 == 1))
                for _ in q00:
                    pass
                for st in (4, 5, 6, 7):
                    spread_v(vp2, st)
                rope_k(1, 0)
                rope_k(1, 1)
                emit_qproj(0, 1)

            # ---------------- phases 1+2 + stage E ------------------------
            wo_pool = stack.enter_context(tc.tile_pool(name=pfx + "wo", bufs=1))
            wobig = wo_pool.tile([P, 4, 8, 512], bf16, tag="wo",
                                 name=pfx + "wo")

            def load_wo(ot):
                nc.sync.dma_start(
                    out=wobig[:, ot, :, :],
                    in_=dram_chunk(woT_d, 0, 8, ot * 512, 512, DIM))

            with tc.tile_pool(name=pfx + "expT", bufs=8) as e_pool, \
                 tc.tile_pool(name=pfx + "normtmp", bufs=3) as n_pool, \
                 tc.tile_pool(name=pfx + "outsb", bufs=4) as ob_pool, \
                 tc.tile_pool(name=pfx + "psumS", bufs=3, space="PSUM") as psS, \
                 tc.tile_pool(name=pfx + "psumO", bufs=3, space="PSUM") as psO:

                def normalize(qt, par, t, op):
                    rcp = n_pool.tile([64, 512], f32, tag="rcp",
                                      name=pfx + f"rcp{qt}_{par}_{t}")
                    nc.vector.reciprocal(rcp[:], op[64:128, :])
                    nc.vector.tensor_mul(
                        o_sb[qt][par * 64:par * 64 + 64,
                                 t * 512:(t + 1) * 512],
                        op[0:64, :], rcp[:])

                def gen_pair_pass(qt, t, filler=None):
                    """Both heads of q-tile qt, sq half t, j-interleaved.

                    Pass t=0 covers sq [0:512] (sk tiles j=0..3); pass t=1
                    covers sq [512:1024] (all 8 sk tiles). Scores run 2
                    j-steps ahead of AV so ACT's exp hides behind PE work.
                    Yields twice: after the first j-step's emission (so the
                    driver can weave the previous pass's tail in) and before
                    its own tail (final AVs + normalize).
                    """
                    g = qt // 2
                    njs = 4 if t == 0 else 8
                    op = {}
                    for par in (0, 1):
                        op[par] = psO.tile([P, 512], f32, tag="op",
                                           name=pfx + f"op{qt}_{par}_{t}")
                    pend = []                # AV queue, 2 j-steps deep
                    for j in range(njs):
                        lo = j * P           # sk tile start (global)
                        ll = max(lo - 512 * t, 0)   # local col of sq>=sk edge
                        new = []
                        for par in (0, 1):
                            qh = q_sb[qt][par * 64:par * 64 + 64, :]
                            kh = kh_ap(g, par)
                            sp = psS.tile([P, 512], f32, tag="sp",
                                          name=pfx + f"sp{qt}_{par}_{t}_{j}")
                            nc.tensor.matmul(
                                sp[:, ll:512], kh[:, lo:lo + P],
                                qh[:, 512 * t + ll:512 * (t + 1)],
                                start=True, stop=True)
                            et = e_pool.tile([P, 512], bf16, tag="et",
                                             name=pfx + f"et{qt}_{par}_{t}_{j}")
                            nc.scalar.activation(et[:, ll:512], sp[:, ll:512],
                                                 EXP, scale=0.125)
                            if lo >= 512 * t:  # diagonal chunk in this pass
                                nc.gpsimd.affine_select(
                                    out=et[:, ll:ll + P], in_=et[:, ll:ll + P],
                                    pattern=[[1, P]], channel_multiplier=-1,
                                    base=0, compare_op=mybir.AluOpType.is_ge,
                                    fill=0.0)
                            new.append((par, j, ll, et))
                        pend.append(new)
                        if j == 0:
                            yield    # (A) driver weaves the previous tail in
                        if len(pend) > 2:
                            for par, jj, jl, et in pend.pop(0):
                                nc.tensor.matmul(
                                    op[par][:, jl:512], vaug[jj][:, g, :],
                                    et[:, jl:512],
                                    start=(jj == 0), stop=(jj == njs - 1))
                        if filler is not None:
                            next(filler, None)
                    yield        # (B) tail ready
                    for step in pend:
                        for par, jj, jl, et in step:
                            nc.tensor.matmul(
                                op[par][:, jl:512], vaug[jj][:, g, :],
                                et[:, jl:512],
                                start=(jj == 0), stop=(jj == njs - 1))
                    for par in (0, 1):
                        normalize(qt, par, t, op[par])
                    if filler is not None:
                        for _ in filler:
                            pass

                def stage_e_chunk(ot, sc, pool, nsplit=1):
                    # nsplit=2: two sequential psum groups so the first
                    # half's copy+store overlaps the second's matmuls
                    pe = pool.tile([P, 512], f32,
                                   tag="q" if pool is psQ else "op",
                                   name=pfx + f"pe{ot}_{sc}")
                    w = 512 // nsplit
                    for h in range(nsplit):
                        cols = slice(h * w, (h + 1) * w)
                        for dt_ in range(8):
                            nc.tensor.matmul(
                                pe[:, cols], o_sb[dt_][:, sc * P:(sc + 1) * P],
                                wobig[:, ot, dt_, h * w:(h + 1) * w],
                                start=(dt_ == 0), stop=(dt_ == 7))
                        dst = out_d[sc * P:(sc + 1) * P,
                                    ot * 512 + h * w:ot * 512 + (h + 1) * w]
                        eng = nc.sync if (sc + h) % 2 else nc.scalar
                        ob = ob_pool.tile([P, w], f32, tag="ob",
                                          name=pfx + f"ob{ot}_{sc}_{h}")
                        nc.vector.tensor_copy(ob[:], pe[:, cols])
                        eng.dma_start(out=dst, in_=ob[:])

                def gen_stage_e_early():
                    # chunks with sc<4 only need the t=0 halves of o_sb;
                    # alternate the freed psQ bank and psO's spare slot
                    n = 0
                    for ot in range(4):
                        for sc in range(4):
                            stage_e_chunk(ot, sc, psQ)
                            n += 1
                            if n % 2 == 0:
                                yield

                # phase 2: q projection blocks between the attention passes;
                # passes software-pipelined (next pass's first scores are
                # emitted before the previous pass's tail AVs + normalize);
                # early stage-E chunks fill the tail pass
                prev = None
                for qt in range(8):
                    for t in (0, 1):
                        if qt < 7:
                            fl = gen_qproj(qt + 1, t)
                        else:
                            fl = None if t == 0 else gen_stage_e_early()
                        g = gen_pair_pass(qt, t, filler=fl)
                        next(g)               # first j-step of this pass
                        if prev is not None:
                            next(prev, None)  # previous pass's tail
                            for _ in prev:
                                pass
                        next(g)               # body through yield (B)
                        prev = g
                    if qt < 4:
                        load_wo(qt)
                for _ in prev:
                    pass

                # ------------- Stage E: output projection (sc >= 4) -------
                for ot in range(4):
                    for sc in range(4, 8):
                        last = (ot == 3 and sc == 7)
                        stage_e_chunk(ot, sc, psO, nsplit=2 if last else 1)

    import concourse.tile as tile_mod
    with tile_mod.TileContext(nc) as tc:
        for rep in range(reps):
            emit(tc, f"r{rep}_" if reps > 1 else "")

    nc.compile()
    return nc


def _get_nc():
    if "nc" not in _CACHE:
        _CACHE["nc"] = build_nc()
    return _CACHE["nc"]


def kernel(**inputs):
    from concourse.bass_utils import run_bass_kernel_spmd
    nc = _get_nc()
    in_maps = host_prep(**inputs)
    res = run_bass_kernel_spmd(nc, in_maps, core_ids=list(range(8)))
    outs = [res.results[c]["out"] for c in range(8)]
    full = np.stack([outs[2 * b] + outs[2 * b + 1] for b in range(B)])
    return full.astype(np.float32)


if __name__ == "__main__":
    nc = build_nc()
    print("build ok")
